# revision 1
# baseline (speedup 1.0000x reference)
"""GraphWaveNet layer on 8 Trainium2 NeuronCores.

Sharding: nodes partitioned across 8 cores (1250 each, padded to 1280).
Each core:
  Phase A: gated causal conv + GCN linear for its node shard, producing
           hw rows [node, (t-major, d-minor) 768] in local DRAM (bf16).
  AllGather: full hw table [10240, 768] bf16 on every core.
  Phase C: per 128-dst window, dma_gather hw[src] rows for incoming edges
           (sorted by dst, self-loops folded in as edges, norm pre-scaled),
           segment-sum via one-hot matmuls into PSUM, then fused
           (t,d)->(c,t) permute + residual(x)+bias epilogue.
"""

import os
import numpy as np

import concourse.bass as bass
import concourse.bacc as bacc
import concourse.mybir as mybir
import concourse.tile as tile
from concourse.bass_utils import run_bass_kernel_spmd

N, C, T, E = 10000, 64, 12, 160000
NCORES = 8
NL = N // NCORES            # 1250 real nodes per core
NLP = 1280                  # padded nodes per core
NG = NCORES * NLP           # 10240 padded global nodes
ROW = C * T                 # 768 floats per hw row, (t-major, d-minor)
COLS = NLP * T              # 15360 (n-major, t-minor) columns
NCH = COLS // 512           # 30 conv chunks
WINS = NLP // 128           # 10 dst windows per core
CALL_SLOTS = int(os.environ.get("KCALL_SLOTS", "8"))  # gather slots (of 128 edges) per dma_gather call

GATH_DT = mybir.dt.bfloat16
GATH_NP = np.dtype("bfloat16") if hasattr(np, "bfloat16") else None

LAST_EXEC_NS = None
LAST_RESULTS = None

_prog_cache = {}


def _build_program(S):
    """S = uniform number of 128-edge slots per dst window (compile-time)."""
    f32 = mybir.dt.float32
    AFT = mybir.ActivationFunctionType
    GTOT = WINS * S * 128
    TOTCH = WINS * S

    nc = bacc.Bacc(None, num_devices=NCORES)
    XT = nc.dram_tensor("xt", [C, COLS], f32, kind="ExternalInput")
    XST = nc.dram_tensor("xst", [C, COLS], f32, kind="ExternalInput")
    XR = nc.dram_tensor("xr", [NLP, ROW], f32, kind="ExternalInput")
    WF = nc.dram_tensor("wf", [2 * C, C], f32, kind="ExternalInput")
    WG = nc.dram_tensor("wg", [2 * C, C], f32, kind="ExternalInput")
    FB = nc.dram_tensor("fb", [C, 1], f32, kind="ExternalInput")
    GB = nc.dram_tensor("gb", [C, 1], f32, kind="ExternalInput")
    GW = nc.dram_tensor("gw", [C, C], f32, kind="ExternalInput")
    IOTA = nc.dram_tensor("iota", [128, 128], f32, kind="ExternalInput")
    IDX = nc.dram_tensor("idx", [128, GTOT // 16], mybir.dt.int16, kind="ExternalInput")
    DS = nc.dram_tensor("ds", [128, TOTCH], f32, kind="ExternalInput")
    NM = nc.dram_tensor("nm", [128, TOTCH], f32, kind="ExternalInput")
    OUT = nc.dram_tensor("out", [NLP, ROW], f32, kind="ExternalOutput")

    with tile.TileContext(nc) as tc:
        with (
            tc.tile_pool(name="dram", bufs=1, space="DRAM") as dram,
            tc.tile_pool(name="const", bufs=1) as cp,
        ):
            hw_local = dram.tile([COLS, C], GATH_DT)
            hw_all = dram.tile([NG, ROW], GATH_DT, addr_space="Shared")

            wf_sb = cp.tile([2 * C, C], f32)
            wg_sb = cp.tile([2 * C, C], f32)
            fb_sb = cp.tile([C, 1], f32)
            gb_sb = cp.tile([C, 1], f32)
            gw_sb = cp.tile([C, C], f32)
            iota_sb = cp.tile([128, 128], f32)
            idx_sb = cp.tile([128, GTOT // 16], mybir.dt.int16)
            ds_sb = cp.tile([128, TOTCH], f32)
            nm_sb = cp.tile([128, TOTCH], f32)
            for t, d in (
                (wf_sb, WF), (wg_sb, WG), (fb_sb, FB), (gb_sb, GB),
                (gw_sb, GW), (iota_sb, IOTA), (idx_sb, IDX), (ds_sb, DS),
                (nm_sb, NM),
            ):
                nc.sync.dma_start(t[:], d[:])

            # ---------------- Phase A: conv + hw + local store ----------------
            with (
                tc.tile_pool(name="xa", bufs=1) as xa,
                tc.tile_pool(name="pa", bufs=3) as pa,
                tc.tile_pool(name="ps_a", bufs=2, space="PSUM") as ps_a,
                tc.tile_pool(name="ps_hw", bufs=4, space="PSUM") as ps_hw,
            ):
                xstack = xa.tile([128, COLS], f32)
                nc.sync.dma_start(xstack[0:C, :], XT[:])
                nc.sync.dma_start(xstack[C : 2 * C, :], XST[:])

                for ch in range(NCH):
                    cs = slice(ch * 512, (ch + 1) * 512)
                    pf = ps_a.tile([C, 512], f32, tag="pf")
                    pg = ps_a.tile([C, 512], f32, tag="pg")
                    nc.tensor.matmul(pf[:], wf_sb[:], xstack[:, cs], start=True, stop=True)
                    nc.tensor.matmul(pg[:], wg_sb[:], xstack[:, cs], start=True, stop=True)
                    fsb = pa.tile([C, 512], f32, tag="f")
                    gsb = pa.tile([C, 512], f32, tag="g")
                    nc.scalar.activation(fsb[:], pf[:], AFT.Tanh, bias=fb_sb[:])
                    nc.scalar.activation(gsb[:], pg[:], AFT.Sigmoid, bias=gb_sb[:])
                    hsb = pa.tile([C, 512], f32, tag="h")
                    nc.vector.tensor_mul(hsb[:], fsb[:], gsb[:])
                    hwsb = pa.tile([128, 4 * C], GATH_DT, tag="hw")
                    for j in range(4):
                        phw = ps_hw.tile([128, C], f32, tag="phw")
                        nc.tensor.matmul(
                            phw[:], hsb[:, j * 128 : (j + 1) * 128], gw_sb[:],
                            start=True, stop=True,
                        )
                        nc.vector.tensor_copy(hwsb[:, j * C : (j + 1) * C], phw[:])
                    dst = hw_local[ch * 512 : (ch + 1) * 512, :].rearrange(
                        "(j p) d -> p j d", p=128
                    )
                    nc.sync.dma_start(dst, hwsb[:].rearrange("p (j d) -> p j d", j=4))

            # ---------------- AllGather hw table ----------------
            nc.gpsimd.collective_compute(
                "AllGather",
                mybir.AluOpType.bypass,
                replica_groups=[list(range(NCORES))],
                ins=[hw_local.opt()],
                outs=[hw_all.opt()],
            )

            # ---------------- Phase C: gather + segment-sum + epilogue --------
            with (
                tc.tile_pool(name="gp", bufs=3) as gp,
                tc.tile_pool(name="ohp", bufs=4) as ohp,
                tc.tile_pool(name="fo", bufs=3) as fo,
                tc.tile_pool(name="ps_c", bufs=2, space="PSUM") as ps_c,
            ):
                ncalls = (S + CALL_SLOTS - 1) // CALL_SLOTS
                gcall = 0
                for w in range(WINS):
                    pagg_a = ps_c.tile([128, 384], f32, tag="pa")
                    pagg_b = ps_c.tile([128, 384], f32, tag="pb")
                    for call in range(ncalls):
                        s0 = call * CALL_SLOTS
                        nsl = min(CALL_SLOTS, S - s0)
                        g0 = (w * S + s0) * 128
                        gt = gp.tile([128, nsl, ROW], GATH_DT, tag="g")
                        nc.gpsimd.dma_gather(
                            gt[:, 0:nsl, :],
                            hw_all[:],
                            idx_sb[:, g0 // 16 : (g0 + nsl * 128) // 16],
                            num_idxs=nsl * 128,
                            num_idxs_reg=nsl * 128,
                            elem_size=ROW,
                        )
                        gcall += 1
                        for s in range(s0, s0 + nsl):
                            q = w * S + s
                            oh = ohp.tile([128, 128], GATH_DT, tag="oh")
                            nc.vector.tensor_scalar(
                                oh[:], iota_sb[:], ds_sb[:, q : q + 1],
                                nm_sb[:, q : q + 1],
                                mybir.AluOpType.is_equal, mybir.AluOpType.mult,
                            )
                            nc.tensor.matmul(
                                pagg_a[:], oh[:], gt[:, s - s0, 0:384],
                                start=(s == 0), stop=(s == S - 1),
                            )
                            nc.tensor.matmul(
                                pagg_b[:], oh[:], gt[:, s - s0, 384:768],
                                start=(s == 0), stop=(s == S - 1),
                            )
                    xrt = fo.tile([128, ROW], f32, tag="xr")
                    nc.sync.dma_start(xrt[:], XR[w * 128 : (w + 1) * 128, :])
                    fin = fo.tile([128, ROW], f32, tag="fin")
                    # out[n, c*12+t] = agg[n, t*64+c] + (x[n,c,t] + gcn_b[c])
                    for half, pag in ((0, pagg_a), (1, pagg_b)):
                        outv = fin[:].rearrange("p (c t) -> p c t", c=C)[
                            :, :, half * 6 : (half + 1) * 6
                        ]
                        inv = pag[:].rearrange("p (t d) -> p d t", t=6)
                        xrv = xrt[:].rearrange("p (c t) -> p c t", c=C)[
                            :, :, half * 6 : (half + 1) * 6
                        ]
                        nc.vector.tensor_tensor(outv, inv, xrv, mybir.AluOpType.add)
                    nc.sync.dma_start(OUT[w * 128 : (w + 1) * 128, :], fin[:])

    nc.compile()
    return nc


def _prep_inputs(x, filter_w, filter_b, gate_w, gate_b, gcn_w, gcn_b, edge_index):
    x = np.ascontiguousarray(x, dtype=np.float32)
    src = np.asarray(edge_index[0], dtype=np.int64)
    dst = np.asarray(edge_index[1], dtype=np.int64)

    deg = (np.bincount(dst, minlength=N) + 1.0).astype(np.float32)
    dinv = (1.0 / np.sqrt(deg)).astype(np.float32)
    norm_e = dinv[src] * dinv[dst]          # [E]
    self_norm = (1.0 / deg).astype(np.float32)

    # padded global src id
    srcg_all = (src // NL) * NLP + (src % NL)

    # per-core edge partitions (by dst), with self-loop edges appended
    per_core = []
    max_slots = 1
    for k in range(NCORES):
        lo, hi = k * NL, (k + 1) * NL
        m = (dst >= lo) & (dst < hi)
        e_src = srcg_all[m]
        e_dstl = (dst[m] - lo).astype(np.int64)
        e_norm = norm_e[m]
        n_ids = np.arange(NL, dtype=np.int64)
        e_src = np.concatenate([e_src, k * NLP + n_ids])
        e_dstl = np.concatenate([e_dstl, n_ids])
        e_norm = np.concatenate([e_norm, self_norm[lo:hi]])
        win = e_dstl // 128
        counts = np.bincount(win, minlength=WINS)
        max_slots = max(max_slots, int(-(-counts.max() // 128)))
        per_core.append((e_src, e_dstl, e_norm, win, counts))

    S = int(max_slots)
    GTOT = WINS * S * 128
    TOTCH = WINS * S

    wf = np.concatenate([filter_w[:, :, 1].T, filter_w[:, :, 0].T]).astype(np.float32)
    wg = np.concatenate([gate_w[:, :, 1].T, gate_w[:, :, 0].T]).astype(np.float32)
    fb = np.asarray(filter_b, np.float32).reshape(C, 1)
    gb = np.asarray(gate_b, np.float32).reshape(C, 1)
    gw = np.ascontiguousarray(gcn_w, np.float32)
    iota = np.broadcast_to(np.arange(128, dtype=np.float32), (128, 128)).copy()
    bias_row = np.repeat(np.asarray(gcn_b, np.float32), T)  # [768] at (c,t)

    in_maps = []
    for k in range(NCORES):
        lo, hi = k * NL, (k + 1) * NL
        xs = x[lo:hi]                                   # [1250, 64, 12]
        xt = np.zeros((C, COLS), np.float32)
        xt[:, : NL * T] = xs.transpose(1, 0, 2).reshape(C, NL * T)
        xshift = np.zeros_like(xs)
        xshift[:, :, 1:] = xs[:, :, :-1]
        xst = np.zeros((C, COLS), np.float32)
        xst[:, : NL * T] = xshift.transpose(1, 0, 2).reshape(C, NL * T)
        xr = np.zeros((NLP, ROW), np.float32)
        xr[:NL] = xs.reshape(NL, ROW) + bias_row[None, :]

        e_src, e_dstl, e_norm, win, counts = per_core[k]
        order = np.argsort(win, kind="stable")
        offs = np.zeros(WINS + 1, np.int64)
        offs[1:] = np.cumsum(counts)
        pos = np.arange(len(order)) - offs[win[order]]
        srcg_pad = np.zeros((WINS, S * 128), np.int64)
        dstsl_pad = np.zeros((WINS, S * 128), np.float32)
        norm_pad = np.zeros((WINS, S * 128), np.float32)
        wi = win[order]
        srcg_pad[wi, pos] = e_src[order]
        dstsl_pad[wi, pos] = (e_dstl[order] % 128).astype(np.float32)
        norm_pad[wi, pos] = e_norm[order]

        flat_src = srcg_pad.reshape(GTOT).astype(np.int16)
        idx16 = np.tile(flat_src.reshape(GTOT // 16, 16).T, (8, 1))  # [128, GTOT//16]
        ds_t = dstsl_pad.reshape(TOTCH, 128).T.copy()
        nm_t = norm_pad.reshape(TOTCH, 128).T.copy()

        in_maps.append({
            "xt": xt, "xst": xst, "xr": xr,
            "wf": wf, "wg": wg, "fb": fb, "gb": gb, "gw": gw,
            "iota": iota, "idx": np.ascontiguousarray(idx16),
            "ds": np.ascontiguousarray(ds_t), "nm": np.ascontiguousarray(nm_t),
        })
    return S, in_maps


def benchmark(x, filter_w, filter_b, gate_w, gate_b, gcn_w, gcn_b, edge_index,
              n_lo=8, n_hi=24):
    """Steady-state per-iteration wall time (ns) with device-resident inputs.

    Replicates bass2jax.run_bass_via_pjrt's plumbing minus output donation so
    the same jitted executable can run repeatedly; differences two iteration
    counts to cancel per-session fixed overhead (axon dispatch remains).
    """
    import time
    import jax
    from jax.experimental.shard_map import shard_map
    from jax.sharding import Mesh, PartitionSpec, NamedSharding
    from concourse import bass2jax as b2j
    import concourse.mybir as mb

    S, in_maps = _prep_inputs(
        x, filter_w, filter_b, gate_w, gate_b, gcn_w, gcn_b, edge_index
    )
    if S not in _prog_cache:
        _prog_cache[S] = _build_program(S)
    nc = _prog_cache[S]
    b2j.install_neuronx_cc_hook()

    in_names, out_names, out_avals, zero_outs = [], [], [], []
    partition_name = nc.partition_id_tensor.name if nc.partition_id_tensor else None
    for alloc in nc.m.functions[0].allocations:
        if not isinstance(alloc, mb.MemoryLocationSet):
            continue
        name = alloc.memorylocations[0].name
        if alloc.kind == "ExternalInput":
            if name != partition_name:
                in_names.append(name)
        elif alloc.kind == "ExternalOutput":
            out_names.append(name)
            shape = tuple(alloc.tensor_shape)
            dtype = mb.dt.np(alloc.dtype)
            out_avals.append(jax.core.ShapedArray(shape, dtype))
            zero_outs.append(np.zeros(shape, dtype))
    n_params = len(in_names)
    all_names = in_names + out_names
    if partition_name is not None:
        all_names.append(partition_name)

    def _body(*args):
        operands = list(args)
        if partition_name is not None:
            operands.append(b2j.partition_id_tensor())
        return tuple(b2j._bass_exec_p.bind(
            *operands,
            out_avals=tuple(out_avals),
            in_names=tuple(all_names),
            out_names=tuple(out_names),
            lowering_input_output_aliases=(),
            sim_require_finite=True,
            sim_require_nnan=True,
            nc=nc,
        ))

    devices = jax.devices()[:NCORES]
    mesh = Mesh(np.asarray(devices), ("core",))
    nin = n_params + len(zero_outs)
    sharded = jax.jit(
        shard_map(_body, mesh=mesh,
                  in_specs=(PartitionSpec("core"),) * nin,
                  out_specs=(PartitionSpec("core"),) * len(out_names),
                  check_rep=False),
        keep_unused=True,
    )
    sh = NamedSharding(mesh, PartitionSpec("core"))
    args = [
        jax.device_put(
            np.concatenate([np.asarray(in_maps[c][n]) for c in range(NCORES)], 0), sh)
        for n in in_names
    ] + [
        jax.device_put(np.zeros((NCORES * z.shape[0], *z.shape[1:]), z.dtype), sh)
        for z in zero_outs
    ]

    def run(n):
        t0 = time.perf_counter()
        outs = None
        for _ in range(n):
            outs = sharded(*args)
        jax.block_until_ready(outs)
        return (time.perf_counter() - t0) * 1e9

    run(6)  # warmup
    ests = []
    for _ in range(2):
        t_lo = run(40)
        t_hi = run(120)
        ests.append((t_hi - t_lo) / 80)
    return min(ests), max(ests)


def kernel(x, filter_w, filter_b, gate_w, gate_b, gcn_w, gcn_b, edge_index):
    global LAST_EXEC_NS, LAST_RESULTS
    S, in_maps = _prep_inputs(
        x, filter_w, filter_b, gate_w, gate_b, gcn_w, gcn_b, edge_index
    )
    if S not in _prog_cache:
        _prog_cache[S] = _build_program(S)
    nc = _prog_cache[S]

    trace = os.environ.get("KBENCH_TRACE", "0") == "1"
    res = run_bass_kernel_spmd(
        nc, in_maps, core_ids=list(range(NCORES)), trace=trace,
        trace_cores=list(range(NCORES)) if trace else None,
    )
    LAST_EXEC_NS = res.exec_time_ns
    LAST_RESULTS = res
    out = np.empty((N, C, T), np.float32)
    for k in range(NCORES):
        rows = res.results[k]["out"][:NL]         # [1250, 768] (c-major, t-minor)
        out[k * NL : (k + 1) * NL] = rows.reshape(NL, C, T)
    return out



# revision 3
# speedup vs baseline: 2.2185x; 2.2185x over previous
"""GraphWaveNet layer on 8 Trainium2 NeuronCores.

Sharding: nodes partitioned across 8 cores (1250 each, padded to 1280).
Each core:
  Phase A: gated causal conv + GCN linear for its node shard, producing
           hw rows [node, (t-major, d-minor) 768] in local DRAM (bf16).
  AllGather: full hw table [10240, 768] bf16 on every core.
  Phase C: per 128-dst window, dma_gather hw[src] rows for incoming edges
           (sorted by dst, self-loops folded in as edges, norm pre-scaled),
           segment-sum via one-hot matmuls into PSUM, then fused
           (t,d)->(c,t) permute + residual(x)+bias epilogue.
"""

import os
import numpy as np

import concourse.bass as bass
import concourse.bacc as bacc
import concourse.mybir as mybir
import concourse.tile as tile
from concourse.bass_utils import run_bass_kernel_spmd

N, C, T, E = 10000, 64, 12, 160000
NCORES = 8
NL = N // NCORES            # 1250 real nodes per core
NLP = 1280                  # padded nodes per core
NG = NCORES * NLP           # 10240 padded global nodes
ROW = C * T                 # 768 floats per hw row, (t-major, d-minor)
COLS = NLP * T              # 15360 (n-major, t-minor) columns
NCH = COLS // 512           # 30 conv chunks
WINS = NLP // 128           # 10 dst windows per core
CALL_SLOTS = int(os.environ.get("KCALL_SLOTS", "8"))  # gather slots (of 128 edges) per dma_gather call

GATH_DT = mybir.dt.bfloat16
GATH_NP = np.dtype("bfloat16") if hasattr(np, "bfloat16") else None

LAST_EXEC_NS = None
LAST_RESULTS = None

_prog_cache = {}


def _build_program(S):
    """S = uniform number of 128-edge slots per dst window (compile-time)."""
    f32 = mybir.dt.float32
    AFT = mybir.ActivationFunctionType
    GTOT = WINS * S * 128
    TOTCH = WINS * S

    nc = bacc.Bacc(None, num_devices=NCORES)
    XT = nc.dram_tensor("xt", [C, COLS], f32, kind="ExternalInput")
    XST = nc.dram_tensor("xst", [C, COLS], f32, kind="ExternalInput")
    XR = nc.dram_tensor("xr", [NLP, ROW], f32, kind="ExternalInput")
    WF = nc.dram_tensor("wf", [2 * C, C], f32, kind="ExternalInput")
    WG = nc.dram_tensor("wg", [2 * C, C], f32, kind="ExternalInput")
    FB = nc.dram_tensor("fb", [C, 1], f32, kind="ExternalInput")
    GB = nc.dram_tensor("gb", [C, 1], f32, kind="ExternalInput")
    GW = nc.dram_tensor("gw", [C, C], f32, kind="ExternalInput")
    IOTA = nc.dram_tensor("iota", [128, 128], f32, kind="ExternalInput")
    IDX = nc.dram_tensor("idx", [128, GTOT // 16], mybir.dt.int16, kind="ExternalInput")
    DS = nc.dram_tensor("ds", [128, TOTCH], f32, kind="ExternalInput")
    NM = nc.dram_tensor("nm", [128, TOTCH], f32, kind="ExternalInput")
    OUT = nc.dram_tensor("out", [NLP, ROW], f32, kind="ExternalOutput")

    with tile.TileContext(nc) as tc:
        with (
            tc.tile_pool(name="dram", bufs=1, space="DRAM") as dram,
            tc.tile_pool(name="const", bufs=1) as cp,
        ):
            hw_local = dram.tile([COLS, C], GATH_DT)
            hw_all = dram.tile([NG, ROW], GATH_DT, addr_space="Shared")

            wf_sb = cp.tile([2 * C, C], f32)
            wg_sb = cp.tile([2 * C, C], f32)
            fb_sb = cp.tile([C, 1], f32)
            gb_sb = cp.tile([C, 1], f32)
            gw_sb = cp.tile([C, C], f32)
            iota_sb = cp.tile([128, 128], f32)
            idx_sb = cp.tile([128, GTOT // 16], mybir.dt.int16)
            ds_sb = cp.tile([128, TOTCH], f32)
            nm_sb = cp.tile([128, TOTCH], f32)
            for t, d in (
                (wf_sb, WF), (wg_sb, WG), (fb_sb, FB), (gb_sb, GB),
                (gw_sb, GW), (iota_sb, IOTA), (idx_sb, IDX), (ds_sb, DS),
                (nm_sb, NM),
            ):
                nc.sync.dma_start(t[:], d[:])

            # ---------------- Phase A: conv + hw + local store ----------------
            with (
                tc.tile_pool(name="xa", bufs=1) as xa,
                tc.tile_pool(name="pa", bufs=3) as pa,
                tc.tile_pool(name="ps_a", bufs=2, space="PSUM") as ps_a,
                tc.tile_pool(name="ps_hw", bufs=4, space="PSUM") as ps_hw,
            ):
                xstack = xa.tile([128, COLS], f32)
                nc.sync.dma_start(xstack[0:C, :], XT[:])
                nc.sync.dma_start(xstack[C : 2 * C, :], XST[:])

                for ch in range(NCH):
                    cs = slice(ch * 512, (ch + 1) * 512)
                    pf = ps_a.tile([C, 512], f32, tag="pf")
                    pg = ps_a.tile([C, 512], f32, tag="pg")
                    nc.tensor.matmul(pf[:], wf_sb[:], xstack[:, cs], start=True, stop=True)
                    nc.tensor.matmul(pg[:], wg_sb[:], xstack[:, cs], start=True, stop=True)
                    fsb = pa.tile([C, 512], f32, tag="f")
                    gsb = pa.tile([C, 512], f32, tag="g")
                    nc.scalar.activation(fsb[:], pf[:], AFT.Tanh, bias=fb_sb[:])
                    nc.scalar.activation(gsb[:], pg[:], AFT.Sigmoid, bias=gb_sb[:])
                    hsb = pa.tile([C, 512], f32, tag="h")
                    nc.vector.tensor_mul(hsb[:], fsb[:], gsb[:])
                    hwsb = pa.tile([128, 4 * C], GATH_DT, tag="hw")
                    for j in range(4):
                        phw = ps_hw.tile([128, C], f32, tag="phw")
                        nc.tensor.matmul(
                            phw[:], hsb[:, j * 128 : (j + 1) * 128], gw_sb[:],
                            start=True, stop=True,
                        )
                        nc.vector.tensor_copy(hwsb[:, j * C : (j + 1) * C], phw[:])
                    dst = hw_local[ch * 512 : (ch + 1) * 512, :].rearrange(
                        "(j p) d -> p j d", p=128
                    )
                    nc.sync.dma_start(dst, hwsb[:].rearrange("p (j d) -> p j d", j=4))

            # ---------------- AllGather hw table ----------------
            nc.gpsimd.collective_compute(
                "AllGather",
                mybir.AluOpType.bypass,
                replica_groups=[list(range(NCORES))],
                ins=[hw_local.opt()],
                outs=[hw_all.opt()],
            )

            # ---------------- Phase C: gather + segment-sum + epilogue --------
            with (
                tc.tile_pool(name="gp", bufs=3) as gp,
                tc.tile_pool(name="ohp", bufs=4) as ohp,
                tc.tile_pool(name="fo", bufs=3) as fo,
                tc.tile_pool(name="ps_c", bufs=2, space="PSUM") as ps_c,
            ):
                ncalls = (S + CALL_SLOTS - 1) // CALL_SLOTS
                gcall = 0
                for w in range(WINS):
                    pagg_a = ps_c.tile([128, 384], f32, tag="pa")
                    pagg_b = ps_c.tile([128, 384], f32, tag="pb")
                    for call in range(ncalls):
                        s0 = call * CALL_SLOTS
                        nsl = min(CALL_SLOTS, S - s0)
                        g0 = (w * S + s0) * 128
                        gt = gp.tile([128, nsl, ROW], GATH_DT, tag="g")
                        nc.gpsimd.dma_gather(
                            gt[:, 0:nsl, :],
                            hw_all[:],
                            idx_sb[:, g0 // 16 : (g0 + nsl * 128) // 16],
                            num_idxs=nsl * 128,
                            num_idxs_reg=nsl * 128,
                            elem_size=ROW,
                        )
                        gcall += 1
                        for s in range(s0, s0 + nsl):
                            q = w * S + s
                            oh = ohp.tile([128, 128], GATH_DT, tag="oh")
                            nc.vector.tensor_scalar(
                                oh[:], iota_sb[:], ds_sb[:, q : q + 1],
                                nm_sb[:, q : q + 1],
                                mybir.AluOpType.is_equal, mybir.AluOpType.mult,
                            )
                            nc.tensor.matmul(
                                pagg_a[:], oh[:], gt[:, s - s0, 0:384],
                                start=(s == 0), stop=(s == S - 1),
                            )
                            nc.tensor.matmul(
                                pagg_b[:], oh[:], gt[:, s - s0, 384:768],
                                start=(s == 0), stop=(s == S - 1),
                            )
                    xrt = fo.tile([128, ROW], f32, tag="xr")
                    nc.sync.dma_start(xrt[:], XR[w * 128 : (w + 1) * 128, :])
                    fin = fo.tile([128, ROW], f32, tag="fin")
                    # out[n, c*12+t] = agg[n, t*64+c] + (x[n,c,t] + gcn_b[c])
                    for half, pag in ((0, pagg_a), (1, pagg_b)):
                        outv = fin[:].rearrange("p (c t) -> p c t", c=C)[
                            :, :, half * 6 : (half + 1) * 6
                        ]
                        inv = pag[:].rearrange("p (t d) -> p d t", t=6)
                        xrv = xrt[:].rearrange("p (c t) -> p c t", c=C)[
                            :, :, half * 6 : (half + 1) * 6
                        ]
                        nc.vector.tensor_tensor(outv, inv, xrv, mybir.AluOpType.add)
                    nc.sync.dma_start(OUT[w * 128 : (w + 1) * 128, :], fin[:])

    nc.compile()
    return nc


def _prep_inputs(x, filter_w, filter_b, gate_w, gate_b, gcn_w, gcn_b, edge_index):
    x = np.ascontiguousarray(x, dtype=np.float32)
    src = np.asarray(edge_index[0], dtype=np.int64)
    dst = np.asarray(edge_index[1], dtype=np.int64)

    deg = (np.bincount(dst, minlength=N) + 1.0).astype(np.float32)
    dinv = (1.0 / np.sqrt(deg)).astype(np.float32)
    norm_e = dinv[src] * dinv[dst]          # [E]
    self_norm = (1.0 / deg).astype(np.float32)

    # padded global src id
    srcg_all = (src // NL) * NLP + (src % NL)

    # per-core edge partitions (by dst), with self-loop edges appended
    per_core = []
    max_slots = 1
    for k in range(NCORES):
        lo, hi = k * NL, (k + 1) * NL
        m = (dst >= lo) & (dst < hi)
        e_src = srcg_all[m]
        e_dstl = (dst[m] - lo).astype(np.int64)
        e_norm = norm_e[m]
        n_ids = np.arange(NL, dtype=np.int64)
        e_src = np.concatenate([e_src, k * NLP + n_ids])
        e_dstl = np.concatenate([e_dstl, n_ids])
        e_norm = np.concatenate([e_norm, self_norm[lo:hi]])
        win = e_dstl // 128
        counts = np.bincount(win, minlength=WINS)
        max_slots = max(max_slots, int(-(-counts.max() // 128)))
        per_core.append((e_src, e_dstl, e_norm, win, counts))

    S = int(max_slots)
    GTOT = WINS * S * 128
    TOTCH = WINS * S

    wf = np.concatenate([filter_w[:, :, 1].T, filter_w[:, :, 0].T]).astype(np.float32)
    wg = np.concatenate([gate_w[:, :, 1].T, gate_w[:, :, 0].T]).astype(np.float32)
    fb = np.asarray(filter_b, np.float32).reshape(C, 1)
    gb = np.asarray(gate_b, np.float32).reshape(C, 1)
    gw = np.ascontiguousarray(gcn_w, np.float32)
    iota = np.broadcast_to(np.arange(128, dtype=np.float32), (128, 128)).copy()
    bias_row = np.repeat(np.asarray(gcn_b, np.float32), T)  # [768] at (c,t)

    in_maps = []
    for k in range(NCORES):
        lo, hi = k * NL, (k + 1) * NL
        xs = x[lo:hi]                                   # [1250, 64, 12]
        xt = np.zeros((C, COLS), np.float32)
        xt[:, : NL * T] = xs.transpose(1, 0, 2).reshape(C, NL * T)
        xshift = np.zeros_like(xs)
        xshift[:, :, 1:] = xs[:, :, :-1]
        xst = np.zeros((C, COLS), np.float32)
        xst[:, : NL * T] = xshift.transpose(1, 0, 2).reshape(C, NL * T)
        xr = np.zeros((NLP, ROW), np.float32)
        xr[:NL] = xs.reshape(NL, ROW) + bias_row[None, :]

        e_src, e_dstl, e_norm, win, counts = per_core[k]
        order = np.argsort(win, kind="stable")
        offs = np.zeros(WINS + 1, np.int64)
        offs[1:] = np.cumsum(counts)
        pos = np.arange(len(order)) - offs[win[order]]
        srcg_pad = np.zeros((WINS, S * 128), np.int64)
        dstsl_pad = np.zeros((WINS, S * 128), np.float32)
        norm_pad = np.zeros((WINS, S * 128), np.float32)
        wi = win[order]
        srcg_pad[wi, pos] = e_src[order]
        dstsl_pad[wi, pos] = (e_dstl[order] % 128).astype(np.float32)
        norm_pad[wi, pos] = e_norm[order]

        flat_src = srcg_pad.reshape(GTOT).astype(np.int16)
        idx16 = np.tile(flat_src.reshape(GTOT // 16, 16).T, (8, 1))  # [128, GTOT//16]
        ds_t = dstsl_pad.reshape(TOTCH, 128).T.copy()
        nm_t = norm_pad.reshape(TOTCH, 128).T.copy()

        in_maps.append({
            "xt": xt, "xst": xst, "xr": xr,
            "wf": wf, "wg": wg, "fb": fb, "gb": gb, "gw": gw,
            "iota": iota, "idx": np.ascontiguousarray(idx16),
            "ds": np.ascontiguousarray(ds_t), "nm": np.ascontiguousarray(nm_t),
        })
    return S, in_maps


def benchmark(x, filter_w, filter_b, gate_w, gate_b, gcn_w, gcn_b, edge_index,
              n_lo=8, n_hi=24):
    """Steady-state per-iteration wall time (ns) with device-resident inputs.

    Replicates bass2jax.run_bass_via_pjrt's plumbing minus output donation so
    the same jitted executable can run repeatedly; differences two iteration
    counts to cancel per-session fixed overhead (axon dispatch remains).
    """
    import time
    import jax
    from jax.experimental.shard_map import shard_map
    from jax.sharding import Mesh, PartitionSpec, NamedSharding
    from concourse import bass2jax as b2j
    import concourse.mybir as mb

    S, in_maps = _prep_inputs(
        x, filter_w, filter_b, gate_w, gate_b, gcn_w, gcn_b, edge_index
    )
    if S not in _prog_cache:
        _prog_cache[S] = _build_program(S)
    nc = _prog_cache[S]
    b2j.install_neuronx_cc_hook()

    in_names, out_names, out_avals, zero_outs = [], [], [], []
    partition_name = nc.partition_id_tensor.name if nc.partition_id_tensor else None
    for alloc in nc.m.functions[0].allocations:
        if not isinstance(alloc, mb.MemoryLocationSet):
            continue
        name = alloc.memorylocations[0].name
        if alloc.kind == "ExternalInput":
            if name != partition_name:
                in_names.append(name)
        elif alloc.kind == "ExternalOutput":
            out_names.append(name)
            shape = tuple(alloc.tensor_shape)
            dtype = mb.dt.np(alloc.dtype)
            out_avals.append(jax.core.ShapedArray(shape, dtype))
            zero_outs.append(np.zeros(shape, dtype))
    n_params = len(in_names)
    all_names = in_names + out_names
    if partition_name is not None:
        all_names.append(partition_name)

    def _body(*args):
        operands = list(args)
        if partition_name is not None:
            operands.append(b2j.partition_id_tensor())
        return tuple(b2j._bass_exec_p.bind(
            *operands,
            out_avals=tuple(out_avals),
            in_names=tuple(all_names),
            out_names=tuple(out_names),
            lowering_input_output_aliases=(),
            sim_require_finite=True,
            sim_require_nnan=True,
            nc=nc,
        ))

    devices = jax.devices()[:NCORES]
    mesh = Mesh(np.asarray(devices), ("core",))
    nin = n_params + len(zero_outs)
    sharded = jax.jit(
        shard_map(_body, mesh=mesh,
                  in_specs=(PartitionSpec("core"),) * nin,
                  out_specs=(PartitionSpec("core"),) * len(out_names),
                  check_rep=False),
        keep_unused=True,
    )
    sh = NamedSharding(mesh, PartitionSpec("core"))
    args = [
        jax.device_put(
            np.concatenate([np.asarray(in_maps[c][n]) for c in range(NCORES)], 0), sh)
        for n in in_names
    ] + [
        jax.device_put(np.zeros((NCORES * z.shape[0], *z.shape[1:]), z.dtype), sh)
        for z in zero_outs
    ]

    def run(n):
        t0 = time.perf_counter()
        outs = None
        for _ in range(n):
            outs = sharded(*args)
        jax.block_until_ready(outs)
        return (time.perf_counter() - t0) * 1e9

    run(6)  # warmup
    ests = []
    for _ in range(2):
        t_lo = run(40)
        t_hi = run(120)
        ests.append((t_hi - t_lo) / 80)
    return min(ests), max(ests)


def _install_ntff_shim():
    """bass_utils wants antenv.axon_hooks (absent in this image); rebuild the
    NTFF profile hook via ctypes against libaxon_pjrt.so and inject it."""
    import sys
    import types

    if "antenv.axon_hooks" in sys.modules:
        return
    try:
        sys.path.insert(0, "/root/.axon_site")
        from trn_agent_boot.trn_boot import _ntff_profile_via_ctypes

        hook = _ntff_profile_via_ctypes("/opt/axon/libaxon_pjrt.so")
        mod = types.ModuleType("antenv.axon_hooks")
        mod.get_axon_ntff_profile_hook = lambda: hook
        mod.set_axon_ntff_profile_hook = lambda h: None
        import antenv  # noqa: F401  (ensure parent package importable)

        sys.modules["antenv.axon_hooks"] = mod
    except Exception as e:  # pragma: no cover - profiling is best-effort
        print(f"ntff shim failed: {e}", file=sys.stderr)


def kernel(x, filter_w, filter_b, gate_w, gate_b, gcn_w, gcn_b, edge_index):
    global LAST_EXEC_NS, LAST_RESULTS
    S, in_maps = _prep_inputs(
        x, filter_w, filter_b, gate_w, gate_b, gcn_w, gcn_b, edge_index
    )
    if S not in _prog_cache:
        _prog_cache[S] = _build_program(S)
    nc = _prog_cache[S]

    trace = os.environ.get("KBENCH_TRACE", "0") == "1"
    if trace:
        _install_ntff_shim()
    res = run_bass_kernel_spmd(
        nc, in_maps, core_ids=list(range(NCORES)), trace=trace,
        trace_cores=list(range(NCORES)) if trace else None,
    )
    LAST_EXEC_NS = res.exec_time_ns
    LAST_RESULTS = res
    out = np.empty((N, C, T), np.float32)
    for k in range(NCORES):
        rows = res.results[k]["out"][:NL]         # [1250, 768] (c-major, t-minor)
        out[k * NL : (k + 1) * NL] = rows.reshape(NL, C, T)
    return out



# revision 14
# speedup vs baseline: 2.5349x; 1.1426x over previous
"""GraphWaveNet layer on 8 Trainium2 NeuronCores.

Sharding: nodes partitioned across 8 cores (1250 each, padded to 1280).
Per core:
  Phase A: gated causal conv (sigmoid folded into a single tanh via
           sigma(x) = 0.5(1+tanh(x/2)), 0.5's folded into host weights)
           + GCN linear, producing the hw slab [1280 nodes, 768] fp8
           assembled node-contiguously in SBUF, then one DMA to DRAM.
  AllGather: full fp8 hw table [10240, 768] on every core.
  Phase C: per 128-dst window, one dma_gather of hw[src] rows for the
           window's (dst-sorted, padded) edges; segment-sum via fp8
           DoubleRow one-hot matmuls (one-hot & norm precomputed on the
           host) into PSUM; fused permute + residual + bias epilogue.
"""

import os
import numpy as np
import ml_dtypes

import concourse.bass as bass
import concourse.bacc as bacc
import concourse.mybir as mybir
import concourse.tile as tile
from concourse.bass_utils import run_bass_kernel_spmd

N, C, T, E = 10000, 64, 12, 160000
NCORES = 8
NL = N // NCORES            # 1250 real nodes per core
NLP = 1280                  # padded nodes per core
NG = NCORES * NLP           # 10240 padded global nodes
ROW = C * T                 # 768 elems per hw row, (t-major, d-minor)
COLS = NLP * T              # 15360 cols, (t-major, n-minor): col = t*1280 + n
WINS = NLP // 128           # 10 dst windows per core
NGRP = NLP // 128           # 10 node groups per core

F32 = mybir.dt.float32
BF16 = mybir.dt.bfloat16
FP8 = mybir.dt.float8e4
NP_BF16 = ml_dtypes.bfloat16
NP_FP8 = ml_dtypes.float8_e4m3

LAST_EXEC_NS = None
LAST_RESULTS = None

DOUBLE_ROW = os.environ.get("KDOUBLEROW", "1") == "1"
TAB_FP8 = os.environ.get("KTAB", "fp8") == "fp8"      # slab/hw_all/gt dtype
OH_FP8 = os.environ.get("KOH", "fp8") == "fp8"        # one-hot dtype
XR_BF16 = os.environ.get("KXR", "bf16") == "bf16"     # residual dtype
TAB_DT = FP8 if TAB_FP8 else BF16
OH_DT = FP8 if OH_FP8 else BF16
XR_DT = BF16 if XR_BF16 else F32
NP_TAB = NP_FP8 if TAB_FP8 else NP_BF16
NP_OH = NP_FP8 if OH_FP8 else NP_BF16
NP_XR = NP_BF16 if XR_BF16 else __import__("numpy").float32

_prog_cache = {}


def _build_program(S):
    """S = uniform (even) number of 128-edge slots per dst window."""
    AFT = mybir.ActivationFunctionType
    GTOT = WINS * S * 128

    nc = bacc.Bacc(None, num_devices=NCORES)
    XS = nc.dram_tensor("xs", [128, COLS], BF16, kind="ExternalInput")
    WF = nc.dram_tensor("wf", [2 * C, C], BF16, kind="ExternalInput")
    WG = nc.dram_tensor("wg", [2 * C, C], BF16, kind="ExternalInput")
    BIA = nc.dram_tensor("bia", [128, 1], F32, kind="ExternalInput")
    GW = nc.dram_tensor("gw", [2 * C, C], BF16, kind="ExternalInput")
    IDX = nc.dram_tensor("idx", [128, GTOT // 16], mybir.dt.int16, kind="ExternalInput")
    OH = nc.dram_tensor("oh", [128, GTOT], OH_DT, kind="ExternalInput")
    XR = nc.dram_tensor("xr", [NLP, ROW], XR_DT, kind="ExternalInput")
    OUT = nc.dram_tensor("out", [NLP, ROW], F32, kind="ExternalOutput")

    with tile.TileContext(nc) as tc:
        with (
            tc.tile_pool(name="dram", bufs=1, space="DRAM") as dram,
            tc.tile_pool(name="const", bufs=1) as cp,
        ):
            slab_dram = dram.tile([NLP, ROW], TAB_DT)
            hw_all = dram.tile([NG, ROW], TAB_DT, addr_space="Shared")

            # xs gates compute: load it first, in two halves so the first
            # conv chunks start after ~half the transfer.
            xs_sb = cp.tile([128, COLS], BF16)
            nc.sync.dma_start(xs_sb[:, 0 : COLS // 2], XS[:, 0 : COLS // 2])
            nc.sync.dma_start(xs_sb[:, COLS // 2 :], XS[:, COLS // 2 :])

            wf_sb = cp.tile([2 * C, C], BF16)
            wg_sb = cp.tile([2 * C, C], BF16)
            bia_sb = cp.tile([128, 1], F32)
            gw_sb = cp.tile([2 * C, C], BF16)
            idx_sb = cp.tile([128, GTOT // 16], mybir.dt.int16)
            oh_sb = cp.tile([128, GTOT], OH_DT)
            for t, d in (
                (wf_sb, WF), (wg_sb, WG), (bia_sb, BIA), (gw_sb, GW),
                (idx_sb, IDX), (oh_sb, OH),
            ):
                nc.sync.dma_start(t[:], d[:])

            slab_sb = cp.tile([128, NGRP * ROW], TAB_DT)

            # ---------------- Phase A: conv + gcn linear -> slab ------------
            with (
                tc.tile_pool(name="pa", bufs=3) as pa,
                tc.tile_pool(name="ps_fg", bufs=2, space="PSUM") as ps_fg,
                tc.tile_pool(name="ps_hw", bufs=3, space="PSUM") as ps_hw,
            ):
                for t in range(T):
                    for n0, w in ((0, 512), (512, 512), (1024, 256)):
                        cs = slice(t * NLP + n0, t * NLP + n0 + w)
                        nj = w // 128
                        pfg = ps_fg.tile([128, w], F32, tag="fg")
                        nc.tensor.matmul(pfg[0:C, :], wf_sb[:], xs_sb[:, cs],
                                         start=True, stop=True)
                        nc.tensor.matmul(pfg[C:, :], wg_sb[:], xs_sb[:, cs],
                                         start=True, stop=True)
                        # sigma(z)=0.5(1+tanh(z/2)), 0.5s folded into host
                        # weights:  h = tf + tf*tg  contracted against [W; W]
                        # (gw_sb is the stacked [128, 64] matrix), so no
                        # separate add is needed. tg goes to PSUM so the
                        # multiply reads one SBUF + one PSUM operand (walrus
                        # rejects two SBUF inputs at different partitions and
                        # two PSUM inputs).
                        u = pa.tile([128, w], BF16, tag="u")
                        nc.scalar.activation(u[0:C, :], pfg[0:C, :], AFT.Tanh,
                                             bias=bia_sb[0:C, :])
                        tg = ps_fg.tile([C, w], F32, tag="tg")
                        nc.scalar.activation(tg[:], pfg[C:, :], AFT.Tanh,
                                             bias=bia_sb[C:, :])
                        nc.vector.tensor_mul(u[C:, :], u[0:C, :], tg[:])
                        phw = ps_hw.tile([128, nj * C], F32, tag="phw")
                        for j in range(nj):
                            nc.tensor.matmul(
                                phw[:, j * C : (j + 1) * C],
                                u[:, j * 128 : (j + 1) * 128], gw_sb[:],
                                start=True, stop=True,
                            )
                        g0 = n0 // 128
                        dst = slab_sb[:].rearrange("p (g d) -> p g d", d=ROW)[
                            :, g0 : g0 + nj, t * C : (t + 1) * C
                        ]
                        nc.vector.tensor_copy(
                            dst, phw[:].rearrange("p (g d) -> p g d", g=nj)
                        )
            nc.sync.dma_start(
                slab_dram[:].rearrange("(g p) d -> p g d", p=128),
                slab_sb[:].rearrange("p (g d) -> p g d", d=ROW),
            )

            # ---------------- AllGather fp8 hw table ------------------------
            nc.gpsimd.collective_compute(
                "AllGather",
                mybir.AluOpType.bypass,
                replica_groups=[list(range(NCORES))],
                ins=[slab_dram.opt()],
                outs=[hw_all.opt()],
            )

            # ---------------- Phase C: gather + segment-sum + epilogue ------
            with (
                tc.tile_pool(name="gp", bufs=2) as gp,
                tc.tile_pool(name="fo", bufs=3) as fo,
                tc.tile_pool(name="ps_c", bufs=2, space="PSUM") as ps_c,
            ):
                CALL_SLOTS = 8  # 8*128 = 1024 descs = SWDGE ring carveout
                for w in range(WINS):
                    gt = gp.tile([128, S, ROW], TAB_DT, tag="g")
                    for s0 in range(0, S, CALL_SLOTS):
                        nsl = min(CALL_SLOTS, S - s0)
                        g0 = (w * S + s0) * 128
                        nc.gpsimd.dma_gather(
                            gt[:, s0 : s0 + nsl, :],
                            hw_all[:],
                            idx_sb[:, g0 // 16 : (g0 + nsl * 128) // 16],
                            num_idxs=nsl * 128,
                            num_idxs_reg=nsl * 128,
                            elem_size=ROW,
                        )
                    pagg_a = ps_c.tile([128, 384], F32, tag="pa")
                    pagg_b = ps_c.tile([128, 384], F32, tag="pb")
                    if DOUBLE_ROW:
                        npair = S // 2
                        for p_ in range(npair):
                            q = (w * S + 2 * p_) * 128
                            oh2 = oh_sb[:, q : q + 256].rearrange(
                                "p (i m) -> p i m", i=2
                            )
                            ss = dict(start=(p_ == 0), stop=(p_ == npair - 1),
                                      perf_mode=mybir.MatmulPerfMode.DoubleRow)
                            nc.tensor.matmul(
                                pagg_a[:], oh2, gt[:, 2 * p_ : 2 * p_ + 2, 0:384], **ss)
                            nc.tensor.matmul(
                                pagg_b[:], oh2, gt[:, 2 * p_ : 2 * p_ + 2, 384:768], **ss)
                    else:
                        for s in range(S):
                            q = (w * S + s) * 128
                            oh1 = oh_sb[:, q : q + 128]
                            ss = dict(start=(s == 0), stop=(s == S - 1))
                            nc.tensor.matmul(
                                pagg_a[:], oh1, gt[:, s, 0:384], **ss)
                            nc.tensor.matmul(
                                pagg_b[:], oh1, gt[:, s, 384:768], **ss)
                    xrt = fo.tile([128, ROW], XR_DT, tag="xr")
                    nc.sync.dma_start(xrt[:], XR[w * 128 : (w + 1) * 128, :])
                    fin = fo.tile([128, ROW], F32, tag="fin")
                    # out[n, c*12+t] = agg[n, t*64+c] + (x[n,c,t] + gcn_b[c])
                    for half, pag in ((0, pagg_a), (1, pagg_b)):
                        outv = fin[:].rearrange("p (c t) -> p c t", c=C)[
                            :, :, half * 6 : (half + 1) * 6
                        ]
                        inv = pag[:].rearrange("p (t d) -> p d t", t=6)
                        xrv = xrt[:].rearrange("p (c t) -> p c t", c=C)[
                            :, :, half * 6 : (half + 1) * 6
                        ]
                        nc.vector.tensor_tensor(outv, inv, xrv, mybir.AluOpType.add)
                    nc.sync.dma_start(OUT[w * 128 : (w + 1) * 128, :], fin[:])

    nc.compile()
    return nc


def _prep_inputs(x, filter_w, filter_b, gate_w, gate_b, gcn_w, gcn_b, edge_index):
    x = np.ascontiguousarray(x, dtype=np.float32)
    src = np.asarray(edge_index[0], dtype=np.int64)
    dst = np.asarray(edge_index[1], dtype=np.int64)

    deg = (np.bincount(dst, minlength=N) + 1.0).astype(np.float32)
    dinv = (1.0 / np.sqrt(deg)).astype(np.float32)
    norm_e = dinv[src] * dinv[dst]          # [E]
    self_norm = (1.0 / deg).astype(np.float32)

    # padded global src id
    srcg_all = (src // NL) * NLP + (src % NL)

    # per-core edge partitions (by dst), with self-loop edges appended
    per_core = []
    max_slots = 1
    for k in range(NCORES):
        lo, hi = k * NL, (k + 1) * NL
        m = (dst >= lo) & (dst < hi)
        e_src = srcg_all[m]
        e_dstl = (dst[m] - lo).astype(np.int64)
        e_norm = norm_e[m]
        n_ids = np.arange(NL, dtype=np.int64)
        e_src = np.concatenate([e_src, k * NLP + n_ids])
        e_dstl = np.concatenate([e_dstl, n_ids])
        e_norm = np.concatenate([e_norm, self_norm[lo:hi]])
        win = e_dstl // 128
        counts = np.bincount(win, minlength=WINS)
        max_slots = max(max_slots, int(-(-counts.max() // 128)))
        per_core.append((e_src, e_dstl, e_norm, win, counts))

    S = int(max_slots)
    S += S % 2  # DoubleRow processes slot pairs
    GTOT = WINS * S * 128

    # conv weights: stacked [current; shifted], sigmoid folded to tanh:
    #   sigma(z) = 0.5 (1 + tanh(z/2))  ->  gate weights/bias scaled by 0.5,
    #   the outer 0.5 folded into gcn_w.
    wf = np.concatenate([filter_w[:, :, 1].T, filter_w[:, :, 0].T]).astype(NP_BF16)
    wg = (0.5 * np.concatenate([gate_w[:, :, 1].T, gate_w[:, :, 0].T])).astype(NP_BF16)
    bia = np.concatenate(
        [np.asarray(filter_b, np.float32), 0.5 * np.asarray(gate_b, np.float32)]
    ).reshape(128, 1).astype(np.float32)
    gw_half = 0.5 * np.ascontiguousarray(gcn_w).astype(np.float32)
    gw = np.concatenate([gw_half, gw_half]).astype(NP_BF16)   # [128, 64]
    bias_row = np.repeat(np.asarray(gcn_b, np.float32), T)  # [768] at (c,t)

    in_maps = []
    for k in range(NCORES):
        lo, hi = k * NL, (k + 1) * NL
        xs_n = x[lo:hi]                                 # [1250, 64, 12]
        # xs: [128, COLS] bf16, rows 0:64 current x, 64:128 shifted x,
        # cols (t-major, n-minor): col = t*1280 + n
        xs = np.zeros((128, COLS), np.float32)
        xt = xs_n.transpose(1, 2, 0)                    # [C, T, 1250]
        xs[:C].reshape(C, T, NLP)[:, :, :NL] = xt
        xs[C:].reshape(C, T, NLP)[:, 1:, :NL] = xt[:, :-1, :]
        xr = np.zeros((NLP, ROW), np.float32)
        xr[:NL] = xs_n.reshape(NL, ROW) + bias_row[None, :]

        e_src, e_dstl, e_norm, win, counts = per_core[k]
        order = np.argsort(win, kind="stable")
        offs = np.zeros(WINS + 1, np.int64)
        offs[1:] = np.cumsum(counts)
        pos = np.arange(len(order)) - offs[win[order]]
        srcg_pad = np.zeros((WINS, S * 128), np.int64)
        dstsl_pad = np.full((WINS, S * 128), -1, np.int64)
        norm_pad = np.zeros((WINS, S * 128), np.float32)
        wi = win[order]
        srcg_pad[wi, pos] = e_src[order]
        dstsl_pad[wi, pos] = e_dstl[order] % 128
        norm_pad[wi, pos] = e_norm[order]

        flat_src = srcg_pad.reshape(GTOT).astype(np.int16)
        idx16 = np.tile(flat_src.reshape(GTOT // 16, 16).T, (8, 1))  # [128, GTOT//16]

        # one-hot table [128, GTOT] fp8: oh[p, q*128 + d] = norm(q*128+p) if
        # dst_slot(q*128+p) == d else 0   (q = global slot index)
        dsl = dstsl_pad.reshape(GTOT // 128, 128)       # [slots, lane]
        nrm = norm_pad.reshape(GTOT // 128, 128)
        ohm = np.zeros((GTOT // 128, 128, 128), np.float32)  # [slot, lane, d]
        sl, ln = np.nonzero(dsl >= 0)
        ohm[sl, ln, dsl[sl, ln]] = nrm[sl, ln]
        oh = np.ascontiguousarray(
            ohm.transpose(1, 0, 2).reshape(128, GTOT)).astype(NP_OH)

        in_maps.append({
            "xs": xs.astype(NP_BF16), "wf": wf, "wg": wg, "bia": bia, "gw": gw,
            "idx": np.ascontiguousarray(idx16), "oh": oh,
            "xr": xr.astype(NP_XR),
        })
    return S, in_maps


def benchmark(x, filter_w, filter_b, gate_w, gate_b, gcn_w, gcn_b, edge_index,
              n_lo=8, n_hi=24):
    """Steady-state per-iteration wall time (ns) with device-resident inputs."""
    import time
    import jax
    from jax.experimental.shard_map import shard_map
    from jax.sharding import Mesh, PartitionSpec, NamedSharding
    from concourse import bass2jax as b2j
    import concourse.mybir as mb

    S, in_maps = _prep_inputs(
        x, filter_w, filter_b, gate_w, gate_b, gcn_w, gcn_b, edge_index
    )
    if S not in _prog_cache:
        _prog_cache[S] = _build_program(S)
    nc = _prog_cache[S]
    b2j.install_neuronx_cc_hook()

    in_names, out_names, out_avals, zero_outs = [], [], [], []
    partition_name = nc.partition_id_tensor.name if nc.partition_id_tensor else None
    for alloc in nc.m.functions[0].allocations:
        if not isinstance(alloc, mb.MemoryLocationSet):
            continue
        name = alloc.memorylocations[0].name
        if alloc.kind == "ExternalInput":
            if name != partition_name:
                in_names.append(name)
        elif alloc.kind == "ExternalOutput":
            out_names.append(name)
            shape = tuple(alloc.tensor_shape)
            dtype = mb.dt.np(alloc.dtype)
            out_avals.append(jax.core.ShapedArray(shape, dtype))
            zero_outs.append(np.zeros(shape, dtype))
    n_params = len(in_names)
    all_names = in_names + out_names
    if partition_name is not None:
        all_names.append(partition_name)

    def _body(*args):
        operands = list(args)
        if partition_name is not None:
            operands.append(b2j.partition_id_tensor())
        return tuple(b2j._bass_exec_p.bind(
            *operands,
            out_avals=tuple(out_avals),
            in_names=tuple(all_names),
            out_names=tuple(out_names),
            lowering_input_output_aliases=(),
            sim_require_finite=True,
            sim_require_nnan=True,
            nc=nc,
        ))

    devices = jax.devices()[:NCORES]
    mesh = Mesh(np.asarray(devices), ("core",))
    nin = n_params + len(zero_outs)
    sharded = jax.jit(
        shard_map(_body, mesh=mesh,
                  in_specs=(PartitionSpec("core"),) * nin,
                  out_specs=(PartitionSpec("core"),) * len(out_names),
                  check_rep=False),
        keep_unused=True,
    )
    sh = NamedSharding(mesh, PartitionSpec("core"))
    args = [
        jax.device_put(
            np.concatenate([np.asarray(in_maps[c][n]) for c in range(NCORES)], 0), sh)
        for n in in_names
    ] + [
        jax.device_put(np.zeros((NCORES * z.shape[0], *z.shape[1:]), z.dtype), sh)
        for z in zero_outs
    ]

    def run(n):
        t0 = time.perf_counter()
        outs = None
        for _ in range(n):
            outs = sharded(*args)
        jax.block_until_ready(outs)
        return (time.perf_counter() - t0) * 1e9

    run(6)  # warmup
    ests = []
    for _ in range(2):
        t_lo = run(40)
        t_hi = run(120)
        ests.append((t_hi - t_lo) / 80)
    return min(ests), max(ests)


def _install_ntff_shim():
    """bass_utils wants antenv.axon_hooks (absent in this image); rebuild the
    NTFF profile hook via ctypes against libaxon_pjrt.so and inject it."""
    import sys
    import types

    if "antenv.axon_hooks" in sys.modules:
        return
    try:
        sys.path.insert(0, "/root/.axon_site")
        from trn_agent_boot.trn_boot import _ntff_profile_via_ctypes

        hook = _ntff_profile_via_ctypes("/opt/axon/libaxon_pjrt.so")
        mod = types.ModuleType("antenv.axon_hooks")
        mod.get_axon_ntff_profile_hook = lambda: hook
        mod.set_axon_ntff_profile_hook = lambda h: None
        import antenv  # noqa: F401  (ensure parent package importable)

        sys.modules["antenv.axon_hooks"] = mod
    except Exception as e:  # pragma: no cover - profiling is best-effort
        print(f"ntff shim failed: {e}", file=sys.stderr)


def kernel(x, filter_w, filter_b, gate_w, gate_b, gcn_w, gcn_b, edge_index):
    global LAST_EXEC_NS, LAST_RESULTS
    S, in_maps = _prep_inputs(
        x, filter_w, filter_b, gate_w, gate_b, gcn_w, gcn_b, edge_index
    )
    if S not in _prog_cache:
        _prog_cache[S] = _build_program(S)
    nc = _prog_cache[S]

    trace = os.environ.get("KBENCH_TRACE", "0") == "1"
    if trace:
        _install_ntff_shim()
    res = run_bass_kernel_spmd(
        nc, in_maps, core_ids=list(range(NCORES)), trace=trace,
        trace_cores=list(range(NCORES)) if trace else None,
    )
    LAST_EXEC_NS = res.exec_time_ns
    LAST_RESULTS = res
    out = np.empty((N, C, T), np.float32)
    for k in range(NCORES):
        rows = res.results[k]["out"][:NL]         # [1250, 768] (c-major, t-minor)
        out[k * NL : (k + 1) * NL] = rows.reshape(NL, C, T)
    return out


# revision 31
# speedup vs baseline: 2.9943x; 1.1813x over previous
"""GraphWaveNet layer on 8 Trainium2 NeuronCores.

Sharding: nodes partitioned across 8 cores (1250 each, padded to 1280).
Per core:
  Phase A: gated causal conv (sigmoid folded into a single tanh via
           sigma(x) = 0.5(1+tanh(x/2)), 0.5's folded into host weights)
           + GCN linear, producing the hw slab [1280 nodes, 768] fp8
           assembled node-contiguously in SBUF, then one DMA to DRAM.
  AllGather: full fp8 hw table [10240, 768] on every core.
  Phase C: per 128-dst window, one dma_gather of hw[src] rows for the
           window's (dst-sorted, padded) edges; segment-sum via fp8
           DoubleRow one-hot matmuls (one-hot & norm precomputed on the
           host) into PSUM; fused permute + residual + bias epilogue.
"""

import os
import numpy as np
import ml_dtypes

import concourse.bass as bass
import concourse.bacc as bacc
import concourse.mybir as mybir
import concourse.tile as tile
from concourse.bass_utils import run_bass_kernel_spmd

N, C, T, E = 10000, 64, 12, 160000
NCORES = 8
NL = N // NCORES            # 1250 real nodes per core
NLP = 1280                  # padded nodes per core
NG = NCORES * NLP           # 10240 padded global nodes
ROW = C * T                 # 768 elems per hw row, (t-major, d-minor)
COLS = NLP * T              # 15360 cols, (t-major, n-minor): col = t*1280 + n
WINS = NLP // 128           # 10 dst windows per core
NGRP = NLP // 128           # 10 node groups per core

F32 = mybir.dt.float32
BF16 = mybir.dt.bfloat16
FP8 = mybir.dt.float8e4
NP_BF16 = ml_dtypes.bfloat16
NP_FP8 = ml_dtypes.float8_e4m3

LAST_EXEC_NS = None
LAST_RESULTS = None

DOUBLE_ROW = os.environ.get("KDOUBLEROW", "1") == "1"
TAB_FP8 = os.environ.get("KTAB", "fp8") == "fp8"      # slab/hw_all/gt dtype
OH_FP8 = os.environ.get("KOH", "fp8") == "fp8"        # one-hot dtype
XR_BF16 = os.environ.get("KXR", "bf16") == "bf16"     # residual dtype
PREP_PIPE = os.environ.get("KPREP", "0") == "1"       # prepare_only gathers
DENSE = os.environ.get("KDENSE", "1") == "1"          # dense adjacency matmul
NPAIR = NG // 256                                     # 40 src row-pair blocks
# dst windows per PSUM pass (2 banks per window: two [128,512] psum tiles)
PASS_WINS = (4, 4, 2)
TAB_DT = FP8 if TAB_FP8 else BF16
OH_DT = FP8 if OH_FP8 else BF16
XR_DT = BF16 if XR_BF16 else F32
NP_TAB = NP_FP8 if TAB_FP8 else NP_BF16
NP_OH = NP_FP8 if OH_FP8 else NP_BF16
NP_XR = NP_BF16 if XR_BF16 else __import__("numpy").float32

_prog_cache = {}


def _build_program(S):
    """S = uniform (even) number of 128-edge slots per dst window."""
    AFT = mybir.ActivationFunctionType
    GTOT = WINS * S * 128

    # 32 KiB SWDGE scratch -> 2048-descriptor ring: two 1024-desc gather
    # preps can be in flight. Dense mode has no SWDGE traffic.
    nc = bacc.Bacc(None, num_devices=NCORES,
                   dynamic_dma_scratch_size=16384 if DENSE else 32768)
    XS = nc.dram_tensor("xs", [128, COLS], BF16, kind="ExternalInput")
    WF = nc.dram_tensor("wf", [2 * C, C], BF16, kind="ExternalInput")
    WG = nc.dram_tensor("wg", [2 * C, C], BF16, kind="ExternalInput")
    BIA = nc.dram_tensor("bia", [128, 1], F32, kind="ExternalInput")
    GW = nc.dram_tensor("gw", [2 * C, C], BF16, kind="ExternalInput")
    if DENSE:
        MD = nc.dram_tensor("md", [NPAIR * 128, 2 * NLP], OH_DT, kind="ExternalInput")
    else:
        IDX = nc.dram_tensor("idx", [128, GTOT // 16], mybir.dt.int16,
                             kind="ExternalInput")
        OH = nc.dram_tensor("oh", [128, GTOT], OH_DT, kind="ExternalInput")
    XR = nc.dram_tensor("xr", [NLP, ROW], XR_DT, kind="ExternalInput")
    OUT = nc.dram_tensor("out", [NLP, ROW], F32, kind="ExternalOutput")

    with tile.TileContext(nc) as tc:
        with (
            tc.tile_pool(name="dram", bufs=1, space="DRAM") as dram,
            tc.tile_pool(name="const", bufs=1) as cp,
            # gp/fo live for the whole kernel: the gather's deferred (DMA)
            # write must not land in SBUF recycled from phase-scoped pools.
            tc.tile_pool(name="gp", bufs=2) as gp,
            tc.tile_pool(name="fo", bufs=3) as fo,
        ):
            slab_dram = dram.tile([NLP, ROW], TAB_DT)
            hw_all = dram.tile([NG, ROW], TAB_DT, addr_space="Shared")

            # xs gates compute: load it first, in two halves so the first
            # conv chunks start after ~half the transfer.
            xs_sb = cp.tile([128, COLS], BF16)
            nc.sync.dma_start(xs_sb[:, 0 : COLS // 2], XS[:, 0 : COLS // 2])
            nc.sync.dma_start(xs_sb[:, COLS // 2 :], XS[:, COLS // 2 :])

            wf_sb = cp.tile([2 * C, C], BF16)
            wg_sb = cp.tile([2 * C, C], BF16)
            bia_sb = cp.tile([128, 1], F32)
            gw_sb = cp.tile([2 * C, C], BF16)
            loads = [(wf_sb, WF), (wg_sb, WG), (bia_sb, BIA), (gw_sb, GW)]
            if not DENSE:
                idx_sb = cp.tile([128, GTOT // 16], mybir.dt.int16)
                oh_sb = cp.tile([128, GTOT], OH_DT)
                loads += [(idx_sb, IDX), (oh_sb, OH)]
            for t, d in loads:
                nc.sync.dma_start(t[:], d[:])

            slab_sb = cp.tile([128, NGRP * ROW], TAB_DT)

            # ---------------- Phase A: conv + gcn linear -> slab ------------
            with (
                tc.tile_pool(name="pa", bufs=3) as pa,
                tc.tile_pool(name="ps_fg", bufs=2, space="PSUM") as ps_fg,
                tc.tile_pool(name="ps_hw", bufs=3, space="PSUM") as ps_hw,
            ):
                for t in range(T):
                    for n0, w in ((0, 512), (512, 512), (1024, 256)):
                        cs = slice(t * NLP + n0, t * NLP + n0 + w)
                        nj = w // 128
                        pfg = ps_fg.tile([128, w], F32, tag="fg")
                        nc.tensor.matmul(pfg[0:C, :], wf_sb[:], xs_sb[:, cs],
                                         start=True, stop=True)
                        nc.tensor.matmul(pfg[C:, :], wg_sb[:], xs_sb[:, cs],
                                         start=True, stop=True)
                        # sigma(z)=0.5(1+tanh(z/2)), 0.5s folded into host
                        # weights:  h = tf + tf*tg  contracted against [W; W]
                        # (gw_sb is the stacked [128, 64] matrix), so no
                        # separate add is needed. tg goes to PSUM so the
                        # multiply reads one SBUF + one PSUM operand (walrus
                        # rejects two SBUF inputs at different partitions and
                        # two PSUM inputs).
                        u = pa.tile([128, w], BF16, tag="u")
                        nc.scalar.activation(u[0:C, :], pfg[0:C, :], AFT.Tanh,
                                             bias=bia_sb[0:C, :])
                        tg = ps_fg.tile([C, w], F32, tag="tg")
                        nc.scalar.activation(tg[:], pfg[C:, :], AFT.Tanh,
                                             bias=bia_sb[C:, :])
                        nc.vector.tensor_mul(u[C:, :], u[0:C, :], tg[:])
                        phw = ps_hw.tile([128, nj * C], F32, tag="phw")
                        for j in range(nj):
                            nc.tensor.matmul(
                                phw[:, j * C : (j + 1) * C],
                                u[:, j * 128 : (j + 1) * 128], gw_sb[:],
                                start=True, stop=True,
                            )
                        g0 = n0 // 128
                        dst = slab_sb[:].rearrange("p (g d) -> p g d", d=ROW)[
                            :, g0 : g0 + nj, t * C : (t + 1) * C
                        ]
                        nc.vector.tensor_copy(
                            dst, phw[:].rearrange("p (g d) -> p g d", g=nj)
                        )
            nc.sync.dma_start(
                slab_dram[:].rearrange("(g p) d -> p g d", p=128),
                slab_sb[:].rearrange("p (g d) -> p g d", d=ROW),
            )

            # ---------------- AllGather fp8 hw table ------------------------
            nc.gpsimd.collective_compute(
                "AllGather",
                mybir.AluOpType.bypass,
                replica_groups=[list(range(NCORES))],
                ins=[slab_dram.opt()],
                outs=[hw_all.opt()],
            )

            # ---------------- Phase C (dense): adjacency-block matmul -------
            # agg[dst] = sum_p sum_i M[2p+i]^T @ hw[2p+i]  per 128-dst window,
            # streaming the whole fp8 table with large sequential DMAs (no
            # SWDGE gather at all) against host-built dense normalized
            # adjacency blocks, DoubleRow fp8 (2 src blocks per matmul).
            if DENSE:
                with (
                    tc.tile_pool(name="mp", bufs=3) as mp,
                    tc.tile_pool(name="ps_c", bufs=1, space="PSUM") as ps_c,
                ):
                    w0 = 0
                    for nw in PASS_WINS:
                        col0 = w0 * 128
                        paggs = [
                            (ps_c.tile([128, 512], F32, tag=f"pa{wi}",
                                       name=f"pagg_a{wi}"),
                             ps_c.tile([128, 512], F32, tag=f"pb{wi}",
                                       name=f"pagg_b{wi}"))
                            for wi in range(nw)
                        ]
                        for p in range(NPAIR):
                            gtp = gp.tile([128, 2, ROW], TAB_DT, tag="gtp")
                            nc.sync.dma_start(
                                gtp[:],
                                hw_all[256 * p : 256 * (p + 1), :].rearrange(
                                    "(i q) d -> q i d", q=128
                                ),
                            )
                            mt = mp.tile([128, 2, nw * 128], OH_DT, tag="mt")
                            nc.sync.dma_start(
                                mt[:],
                                MD[p * 128 : (p + 1) * 128, :].rearrange(
                                    "s (i d) -> s i d", i=2
                                )[:, :, col0 : col0 + nw * 128],
                            )
                            ss = dict(start=(p == 0), stop=(p == NPAIR - 1),
                                      perf_mode=mybir.MatmulPerfMode.DoubleRow)
                            for wi in range(nw):
                                lhsT = mt[:, :, wi * 128 : (wi + 1) * 128]
                                nc.tensor.matmul(
                                    paggs[wi][0][:, 0:384], lhsT,
                                    gtp[:, :, 0:384], **ss)
                                nc.tensor.matmul(
                                    paggs[wi][1][:, 0:384], lhsT,
                                    gtp[:, :, 384:768], **ss)
                        for wi in range(nw):
                            w = w0 + wi
                            xrt = fo.tile([128, ROW], XR_DT, tag="xr")
                            nc.sync.dma_start(xrt[:], XR[w * 128 : (w + 1) * 128, :])
                            fin = fo.tile([128, ROW], F32, tag="fin")
                            for half in (0, 1):
                                outv = fin[:].rearrange("p (c t) -> p c t", c=C)[
                                    :, :, half * 6 : (half + 1) * 6
                                ]
                                inv = paggs[wi][half][:, 0:384].rearrange(
                                    "p (t d) -> p d t", t=6)
                                xrv = xrt[:].rearrange("p (c t) -> p c t", c=C)[
                                    :, :, half * 6 : (half + 1) * 6
                                ]
                                nc.vector.tensor_tensor(
                                    outv, inv, xrv, mybir.AluOpType.add)
                            nc.sync.dma_start(
                                OUT[w * 128 : (w + 1) * 128, :], fin[:])
                        w0 += nw

            # ---------------- Phase C (gather): SWDGE + segment-sum ---------
            if not DENSE:
              with (
                tc.tile_pool(name="ps_c", bufs=2, space="PSUM") as ps_c,
              ):
                CALL_SLOTS = 8  # 8*128 = 1024 descs; ring holds 2 calls
                dma_sem = nc.alloc_semaphore("gather_dma")
                for w in range(WINS):
                    gt = gp.tile([128, S, ROW], TAB_DT, tag="g")
                    for s0 in range(0, S, CALL_SLOTS):
                        nsl = min(CALL_SLOTS, S - s0)
                        g0 = (w * S + s0) * 128
                        kw = (dict(prepare_only=True, sem=dma_sem)
                              if PREP_PIPE else {})
                        nc.gpsimd.dma_gather(
                            gt[:, s0 : s0 + nsl, :],
                            hw_all[:],
                            idx_sb[:, g0 // 16 : (g0 + nsl * 128) // 16],
                            num_idxs=nsl * 128,
                            num_idxs_reg=nsl * 128,
                            elem_size=ROW,
                            **kw,
                        )
                        if PREP_PIPE:
                            nc.gpsimd.trigger_dma(count=None)
                    pagg_a = ps_c.tile([128, 384], F32, tag="pa")
                    pagg_b = ps_c.tile([128, 384], F32, tag="pb")
                    if DOUBLE_ROW:
                        npair = S // 2
                        for p_ in range(npair):
                            q = (w * S + 2 * p_) * 128
                            oh2 = oh_sb[:, q : q + 256].rearrange(
                                "p (i m) -> p i m", i=2
                            )
                            ss = dict(start=(p_ == 0), stop=(p_ == npair - 1),
                                      perf_mode=mybir.MatmulPerfMode.DoubleRow)
                            nc.tensor.matmul(
                                pagg_a[:], oh2, gt[:, 2 * p_ : 2 * p_ + 2, 0:384], **ss)
                            nc.tensor.matmul(
                                pagg_b[:], oh2, gt[:, 2 * p_ : 2 * p_ + 2, 384:768], **ss)
                    else:
                        for s in range(S):
                            q = (w * S + s) * 128
                            oh1 = oh_sb[:, q : q + 128]
                            ss = dict(start=(s == 0), stop=(s == S - 1))
                            nc.tensor.matmul(
                                pagg_a[:], oh1, gt[:, s, 0:384], **ss)
                            nc.tensor.matmul(
                                pagg_b[:], oh1, gt[:, s, 384:768], **ss)
                    xrt = fo.tile([128, ROW], XR_DT, tag="xr")
                    nc.sync.dma_start(xrt[:], XR[w * 128 : (w + 1) * 128, :])
                    fin = fo.tile([128, ROW], F32, tag="fin")
                    # out[n, c*12+t] = agg[n, t*64+c] + (x[n,c,t] + gcn_b[c])
                    for half, pag in ((0, pagg_a), (1, pagg_b)):
                        outv = fin[:].rearrange("p (c t) -> p c t", c=C)[
                            :, :, half * 6 : (half + 1) * 6
                        ]
                        inv = pag[:].rearrange("p (t d) -> p d t", t=6)
                        xrv = xrt[:].rearrange("p (c t) -> p c t", c=C)[
                            :, :, half * 6 : (half + 1) * 6
                        ]
                        nc.vector.tensor_tensor(outv, inv, xrv, mybir.AluOpType.add)
                    nc.sync.dma_start(OUT[w * 128 : (w + 1) * 128, :], fin[:])

    nc.compile()
    return nc


def _prep_inputs(x, filter_w, filter_b, gate_w, gate_b, gcn_w, gcn_b, edge_index):
    x = np.ascontiguousarray(x, dtype=np.float32)
    src = np.asarray(edge_index[0], dtype=np.int64)
    dst = np.asarray(edge_index[1], dtype=np.int64)

    deg = (np.bincount(dst, minlength=N) + 1.0).astype(np.float32)
    dinv = (1.0 / np.sqrt(deg)).astype(np.float32)
    norm_e = dinv[src] * dinv[dst]          # [E]
    self_norm = (1.0 / deg).astype(np.float32)

    # padded global src id
    srcg_all = (src // NL) * NLP + (src % NL)

    # per-core edge partitions (by dst), self-loop edges appended; gather
    # mode dedupes rows per dst window, dense mode builds the full
    # normalized-adjacency block matrix.
    per_core = []
    max_slots = 1
    for k in range(NCORES):
        lo, hi = k * NL, (k + 1) * NL
        m = (dst >= lo) & (dst < hi)
        e_src = srcg_all[m]
        e_dstl = (dst[m] - lo).astype(np.int64)
        e_norm = norm_e[m]
        n_ids = np.arange(NL, dtype=np.int64)
        e_src = np.concatenate([e_src, k * NLP + n_ids])
        e_dstl = np.concatenate([e_dstl, n_ids])
        e_norm = np.concatenate([e_norm, self_norm[lo:hi]])
        if DENSE:
            per_core.append((e_src, e_dstl, e_norm))
            continue
        win = e_dstl // 128
        wdata = []
        for w in range(WINS):
            mw = win == w
            uniq, inv = np.unique(e_src[mw], return_inverse=True)
            max_slots = max(max_slots, int(-(-len(uniq) // 128)))
            wdata.append((uniq, inv, (e_dstl[mw] % 128), e_norm[mw]))
        per_core.append(wdata)

    S = 0 if DENSE else int(max_slots)
    S += S % 2  # DoubleRow processes slot pairs
    GTOT = WINS * S * 128

    # conv weights: stacked [current; shifted], sigmoid folded to tanh:
    #   sigma(z) = 0.5 (1 + tanh(z/2))  ->  gate weights/bias scaled by 0.5,
    #   the outer 0.5 folded into gcn_w.
    wf = np.concatenate([filter_w[:, :, 1].T, filter_w[:, :, 0].T]).astype(NP_BF16)
    wg = (0.5 * np.concatenate([gate_w[:, :, 1].T, gate_w[:, :, 0].T])).astype(NP_BF16)
    bia = np.concatenate(
        [np.asarray(filter_b, np.float32), 0.5 * np.asarray(gate_b, np.float32)]
    ).reshape(128, 1).astype(np.float32)
    gw_half = 0.5 * np.ascontiguousarray(gcn_w).astype(np.float32)
    gw = np.concatenate([gw_half, gw_half]).astype(NP_BF16)   # [128, 64]
    bias_row = np.repeat(np.asarray(gcn_b, np.float32), T)  # [768] at (c,t)

    in_maps = []
    for k in range(NCORES):
        lo, hi = k * NL, (k + 1) * NL
        xs_n = x[lo:hi]                                 # [1250, 64, 12]
        # xs: [128, COLS] bf16, rows 0:64 current x, 64:128 shifted x,
        # cols (t-major, n-minor): col = t*1280 + n
        xs = np.zeros((128, COLS), np.float32)
        xt = xs_n.transpose(1, 2, 0)                    # [C, T, 1250]
        xs[:C].reshape(C, T, NLP)[:, :, :NL] = xt
        xs[C:].reshape(C, T, NLP)[:, 1:, :NL] = xt[:, :-1, :]
        xr = np.zeros((NLP, ROW), np.float32)
        xr[:NL] = xs_n.reshape(NL, ROW) + bias_row[None, :]

        im = {
            "xs": xs.astype(NP_BF16), "wf": wf, "wg": wg, "bia": bia, "gw": gw,
            "xr": xr.astype(NP_XR),
        }
        if DENSE:
            e_src, e_dstl, e_norm = per_core[k]
            densem = np.zeros((NG, NLP), np.float32)
            np.add.at(densem, (e_src, e_dstl), e_norm)
            # MD[(p,s), (i,d)] = densem[(2p+i)*128+s, d]
            md = (densem.reshape(NPAIR, 2, 128, NLP)
                  .transpose(0, 2, 1, 3).reshape(NPAIR * 128, 2 * NLP))
            im["md"] = md.astype(NP_OH)
        else:
            srcg_pad = np.zeros((WINS, S * 128), np.int64)
            ohm = np.zeros((WINS, S * 128, 128), np.float32)  # [w, lane, d]
            for w in range(WINS):
                uniq, inv, dsl, nrm = per_core[k][w]
                srcg_pad[w, : len(uniq)] = uniq
                np.add.at(ohm[w], (inv, dsl), nrm)

            flat_src = srcg_pad.reshape(GTOT).astype(np.int16)
            idx16 = np.tile(flat_src.reshape(GTOT // 16, 16).T, (8, 1))
            # oh[p, (w*S+s)*128 + d] = summed norm of edges whose (deduped)
            # gathered row sits at lane s*128+p of window w
            oh = np.ascontiguousarray(
                ohm.reshape(WINS * S, 128, 128).transpose(1, 0, 2)
                .reshape(128, GTOT)
            ).astype(NP_OH)
            im["idx"] = np.ascontiguousarray(idx16)
            im["oh"] = oh
        in_maps.append(im)
    return S, in_maps


def benchmark(x, filter_w, filter_b, gate_w, gate_b, gcn_w, gcn_b, edge_index,
              n_lo=8, n_hi=24):
    """Steady-state per-iteration wall time (ns) with device-resident inputs."""
    import time
    import jax
    from jax.experimental.shard_map import shard_map
    from jax.sharding import Mesh, PartitionSpec, NamedSharding
    from concourse import bass2jax as b2j
    import concourse.mybir as mb

    S, in_maps = _prep_inputs(
        x, filter_w, filter_b, gate_w, gate_b, gcn_w, gcn_b, edge_index
    )
    if S not in _prog_cache:
        _prog_cache[S] = _build_program(S)
    nc = _prog_cache[S]
    b2j.install_neuronx_cc_hook()

    in_names, out_names, out_avals, zero_outs = [], [], [], []
    partition_name = nc.partition_id_tensor.name if nc.partition_id_tensor else None
    for alloc in nc.m.functions[0].allocations:
        if not isinstance(alloc, mb.MemoryLocationSet):
            continue
        name = alloc.memorylocations[0].name
        if alloc.kind == "ExternalInput":
            if name != partition_name:
                in_names.append(name)
        elif alloc.kind == "ExternalOutput":
            out_names.append(name)
            shape = tuple(alloc.tensor_shape)
            dtype = mb.dt.np(alloc.dtype)
            out_avals.append(jax.core.ShapedArray(shape, dtype))
            zero_outs.append(np.zeros(shape, dtype))
    n_params = len(in_names)
    all_names = in_names + out_names
    if partition_name is not None:
        all_names.append(partition_name)

    def _body(*args):
        operands = list(args)
        if partition_name is not None:
            operands.append(b2j.partition_id_tensor())
        return tuple(b2j._bass_exec_p.bind(
            *operands,
            out_avals=tuple(out_avals),
            in_names=tuple(all_names),
            out_names=tuple(out_names),
            lowering_input_output_aliases=(),
            sim_require_finite=True,
            sim_require_nnan=True,
            nc=nc,
        ))

    devices = jax.devices()[:NCORES]
    mesh = Mesh(np.asarray(devices), ("core",))
    nin = n_params + len(zero_outs)
    sharded = jax.jit(
        shard_map(_body, mesh=mesh,
                  in_specs=(PartitionSpec("core"),) * nin,
                  out_specs=(PartitionSpec("core"),) * len(out_names),
                  check_rep=False),
        keep_unused=True,
    )
    sh = NamedSharding(mesh, PartitionSpec("core"))
    args = [
        jax.device_put(
            np.concatenate([np.asarray(in_maps[c][n]) for c in range(NCORES)], 0), sh)
        for n in in_names
    ] + [
        jax.device_put(np.zeros((NCORES * z.shape[0], *z.shape[1:]), z.dtype), sh)
        for z in zero_outs
    ]

    def run(n):
        t0 = time.perf_counter()
        outs = None
        for _ in range(n):
            outs = sharded(*args)
        jax.block_until_ready(outs)
        return (time.perf_counter() - t0) * 1e9

    run(6)  # warmup
    ests = []
    for _ in range(2):
        t_lo = run(40)
        t_hi = run(120)
        ests.append((t_hi - t_lo) / 80)
    return min(ests), max(ests)


def _install_ntff_shim():
    """bass_utils wants antenv.axon_hooks (absent in this image); rebuild the
    NTFF profile hook via ctypes against libaxon_pjrt.so and inject it."""
    import sys
    import types

    if "antenv.axon_hooks" in sys.modules:
        return
    try:
        sys.path.insert(0, "/root/.axon_site")
        from trn_agent_boot.trn_boot import _ntff_profile_via_ctypes

        hook = _ntff_profile_via_ctypes("/opt/axon/libaxon_pjrt.so")
        mod = types.ModuleType("antenv.axon_hooks")
        mod.get_axon_ntff_profile_hook = lambda: hook
        mod.set_axon_ntff_profile_hook = lambda h: None
        import antenv  # noqa: F401  (ensure parent package importable)

        sys.modules["antenv.axon_hooks"] = mod
    except Exception as e:  # pragma: no cover - profiling is best-effort
        print(f"ntff shim failed: {e}", file=sys.stderr)


def kernel(x, filter_w, filter_b, gate_w, gate_b, gcn_w, gcn_b, edge_index):
    global LAST_EXEC_NS, LAST_RESULTS
    S, in_maps = _prep_inputs(
        x, filter_w, filter_b, gate_w, gate_b, gcn_w, gcn_b, edge_index
    )
    if S not in _prog_cache:
        _prog_cache[S] = _build_program(S)
    nc = _prog_cache[S]

    trace = os.environ.get("KBENCH_TRACE", "0") == "1"
    if trace:
        _install_ntff_shim()
    res = run_bass_kernel_spmd(
        nc, in_maps, core_ids=list(range(NCORES)), trace=trace,
        trace_cores=list(range(NCORES)) if trace else None,
    )
    LAST_EXEC_NS = res.exec_time_ns
    LAST_RESULTS = res
    out = np.empty((N, C, T), np.float32)
    for k in range(NCORES):
        rows = res.results[k]["out"][:NL]         # [1250, 768] (c-major, t-minor)
        out[k * NL : (k + 1) * NL] = rows.reshape(NL, C, T)
    return out
